# revision 105
# baseline (speedup 1.0000x reference)
"""GNN message-passing kernel for Trainium2 (Bass/Tile), 8-core SPMD.

Sharding: edges sharded by receiver (edge/data parallel, no collectives),
with a degree-balanced node -> (core, window, rank) relabeling that minimizes
padded tile count. Host prep (pure indexing, no model FLOPs) expands
nf[senders] into a per-edge bf16 stream so the device never does a DRAM
gather on the sender side.

Per core, per 128-receiver window, per 128-edge tile (one PSUM group):
  ps  = transpose(p2x)            # P2b[r]: window-local ap_gather, then a
                                  # bf16-bitcast identity matmul (1 cyc/row)
      + nf_exp_tile.T @ W1        # bf16, host-expanded nf[senders]
      + ef_tile.T    @ W3         # bf16
  msg = relu(ps)                  # Activation engine, bf16 out
  S   = (iota == rank)            # one-hot, DVE tensor_scalar, bf16
  agg += S.T @ msg                # scatter-sum via matmul, fp32 PSUM
                                  # (agg matmuls pipelined one group behind)
  agg starts as nf[window] (residual) via a bf16 identity matmul;
  out = LayerNorm(agg) streamed out on the Activation DGE queue.
"""

import numpy as np
import ml_dtypes

import concourse.bacc as bacc
import concourse.tile as tile
import concourse.mybir as mybir
import concourse.bass as bass

F32 = mybir.dt.float32
# float32r crashes real TRN2 (walrus codegen bug for f32r weight loads) —
# keep the transpose path in plain fp32.
F32R = mybir.dt.float32
BF16 = mybir.dt.bfloat16
I16 = mybir.dt.int16

BF = ml_dtypes.bfloat16


# ----------------------------------------------------------------------------
# Host-side preparation (indexing / layout only — no model FLOPs)
# ----------------------------------------------------------------------------

def wrap_idx(arr):
    """int16 stream -> [128, L/16] wrapped layout (replicated per 16 rows)."""
    L = arr.shape[0]
    assert L % 16 == 0
    w16 = arr.reshape(-1, 16).T.astype(np.int16)   # [16, L/16]
    return np.ascontiguousarray(np.tile(w16, (8, 1)))


def host_prep(node_features, senders, receivers, edge_features, W, b, ln_w, ln_b,
              n_cores=8):
    N, H = node_features.shape
    E = senders.shape[0]
    assert H == 128
    NPC = N // n_cores                      # nodes per core
    WPC = (NPC + 127) // 128                # windows per core
    NPC_PAD = WPC * 128

    node_features = np.asarray(node_features, np.float32)
    senders = np.asarray(senders, np.int32)
    receivers = np.asarray(receivers, np.int32)
    edge_features = np.asarray(edge_features, np.float32)

    # ---- degree-balanced node -> (core, window-slot, rank) assignment ------
    # Nodes are packed into n_cores*WPC windows of <=128 nodes so that window
    # degree sums are near-equal (greedy, highest degree first), then windows
    # with similar sums are dealt to the same slot across cores.  This
    # minimizes sum_w max_c ceil(cnt/128), i.e. the padded tile count.
    import heapq
    deg = np.bincount(receivers, minlength=N).astype(np.int64)
    W_TOT = n_cores * WPC
    win_of_node = np.empty(N, np.int64)
    nfill = np.zeros(W_TOT, np.int64)
    heap = [(0, wi) for wi in range(W_TOT)]
    heapq.heapify(heap)
    for nd in np.argsort(-deg, kind="stable"):
        s, wi = heapq.heappop(heap)
        win_of_node[nd] = wi
        nfill[wi] += 1
        if nfill[wi] < 128:
            heapq.heappush(heap, (s + int(deg[nd]), wi))
    wsum = np.bincount(win_of_node, weights=deg.astype(np.float64),
                       minlength=W_TOT)
    wsorted = np.argsort(-wsum, kind="stable")
    slot_of_win = np.empty(W_TOT, np.int64)
    core_of_win = np.empty(W_TOT, np.int64)
    slot_of_win[wsorted] = np.arange(W_TOT) // n_cores
    core_of_win[wsorted] = np.arange(W_TOT) % n_cores
    # rank of node within its window
    order_nd = np.argsort(win_of_node, kind="stable")
    rank_of_node = np.empty(N, np.int64)
    wo = win_of_node[order_nd]
    starts = np.concatenate(([0], np.nonzero(np.diff(wo))[0] + 1))
    grp_start = np.repeat(starts, np.diff(np.concatenate((starts, [N]))))
    rank_of_node[order_nd] = np.arange(N) - grp_start

    node_core = core_of_win[win_of_node]
    node_slot = slot_of_win[win_of_node]
    node_rank = rank_of_node

    # ---- pass 1: per-core window counts ------------------------------------
    core_of_edge = node_core[receivers]
    per_core = []
    cnt = np.zeros((n_cores, WPC), np.int64)
    for c in range(n_cores):
        sel = np.nonzero(core_of_edge == c)[0]
        w = node_slot[receivers[sel]]
        order = np.argsort(w, kind="stable")
        sel, w = sel[order], w[order]
        cnt[c] = np.bincount(w, minlength=WPC)
        per_core.append((sel, w))

    T_w = ((cnt.max(axis=0) + 127) // 128).astype(np.int64)  # shared all cores
    NT = int(T_w.sum())
    E_PAD = NT * 128
    tile_base = np.concatenate(([0], np.cumsum(T_w)[:-1]))

    structure = dict(N=N, H=H, E=E, NPC=NPC, WPC=WPC, NPC_PAD=NPC_PAD,
                     NT=NT, E_PAD=E_PAD, T_w=T_w, tile_base=tile_base,
                     node_core=node_core, node_slot=node_slot,
                     node_rank=node_rank)

    iota_row = np.broadcast_to(np.arange(128, dtype=np.float32),
                               (128, 128)).astype(BF)
    # bf16 const blob: W1 | W2 | W3 | iota | ident   [128, 5*128]
    cb_bf = np.concatenate([
        np.ascontiguousarray(W[0:128]).astype(BF),
        np.ascontiguousarray(W[128:256]).astype(BF),
        np.ascontiguousarray(W[256:384]).astype(BF),
        iota_row,
        np.eye(128, dtype=np.float32).astype(BF),
    ], axis=1)
    # f32 const blob: lnw | lnb | b_col   [128, 2*128+1]
    cb_f32 = np.concatenate([
        np.broadcast_to(np.asarray(ln_w, np.float32), (128, 128)),
        np.broadcast_to(np.asarray(ln_b, np.float32), (128, 128)),
        np.asarray(b, np.float32).reshape(128, 1),
    ], axis=1)
    shared = {
        "cb_bf": np.ascontiguousarray(cb_bf),
        "cb_f32": np.ascontiguousarray(cb_f32),
    }

    # ---- pass 2: per-core padded streams ------------------------------------
    in_maps = []
    for c in range(n_cores):
        sel, w = per_core[c]
        Ec = sel.shape[0]
        starts = np.concatenate(([0], np.nonzero(np.diff(w))[0] + 1))
        grp_start = np.repeat(starts, np.diff(np.concatenate((starts, [Ec]))))
        j = np.arange(Ec) - grp_start
        pos = tile_base[w] * 128 + j
        rank = node_rank[receivers[sel]]

        nf_exp = np.zeros((E_PAD, 128), np.float32)
        nf_exp[pos] = node_features[senders[sel]]
        ef_pad = np.zeros((E_PAD, 128), np.float32)
        ef_pad[pos] = edge_features[sel]

        rank_arr = np.full(E_PAD, -1.0, np.float32)
        rank_arr[pos] = rank
        rankT = np.ascontiguousarray(rank_arr.reshape(NT, 128).T)   # [128, NT]

        idx_rank = np.zeros(E_PAD, np.int64)
        idx_rank[pos] = rank

        # nodes of this core, laid out by (slot, rank), feature-major
        mine = np.nonzero(node_core == c)[0]
        rows = node_slot[mine] * 128 + node_rank[mine]
        nfT_loc = np.zeros((128, NPC_PAD), np.float32)
        nfT_loc[:, rows] = node_features[mine].T

        m = dict(shared)
        m.update({
            "nf_expT": np.ascontiguousarray(nf_exp.T).astype(BF),
            "efT": np.ascontiguousarray(ef_pad.T).astype(BF),
            "rankT": rankT,
            "idx_rank": wrap_idx(idx_rank),
            "nfT_loc": np.ascontiguousarray(nfT_loc).astype(BF),
        })
        in_maps.append(m)

    return structure, in_maps


# ----------------------------------------------------------------------------
# Bass kernel builder
# ----------------------------------------------------------------------------

def _emit_ln_store(nc, wtiles, x, eps_sb, lnw_sb, lnb_sb, out_shard, w):
    """LayerNorm(x) * ln_w + ln_b -> out_shard[w*128:(w+1)*128].

    The store goes out on the Activation DGE queue so it cannot head-of-line
    block the SP queue that prefetches the next windows' edge streams.
    """
    stats = wtiles.tile([128, 6], F32, tag="stats")
    nc.vector.bn_stats(out=stats[:], in_=x[:])
    mv = wtiles.tile([128, 2], F32, tag="mv")
    nc.vector.bn_aggr(out=mv[:], in_=stats[:])
    sd = wtiles.tile([128, 1], F32, tag="sd")
    nc.scalar.activation(
        out=sd[:], in_=mv[:, 1:2],
        func=mybir.ActivationFunctionType.Sqrt,
        bias=eps_sb[:], scale=1.0)
    rs = wtiles.tile([128, 1], F32, tag="rs")
    nc.vector.reciprocal(out=rs[:], in_=sd[:])
    xn = wtiles.tile([128, 128], F32, tag="xn")
    nc.vector.tensor_scalar(
        out=xn[:], in0=x[:], scalar1=mv[:, 0:1], scalar2=rs[:],
        op0=mybir.AluOpType.subtract, op1=mybir.AluOpType.mult)
    xw = wtiles.tile([128, 128], F32, tag="xw")
    nc.vector.tensor_mul(out=xw[:], in0=xn[:], in1=lnw_sb)
    ot = wtiles.tile([128, 128], BF16, tag="ot")
    nc.vector.tensor_add(out=ot[:], in0=xw[:], in1=lnb_sb)
    nc.scalar.dma_start(out=out_shard[w * 128:(w + 1) * 128, :], in_=ot[:])


def build_kernel(st, eps=1e-5, max_windows=None):
    NPC_PAD, WPC = st["NPC_PAD"], st["WPC"]
    NT, E_PAD = st["NT"], st["E_PAD"]
    T_w, tile_base = st["T_w"], st["tile_base"]
    T_MAX = int(T_w.max())
    is_eq = mybir.AluOpType.is_equal

    nc = bacc.Bacc("TRN2", target_bir_lowering=False, debug=False)

    # inputs
    nf_expT = nc.dram_tensor("nf_expT", [128, E_PAD], BF16, kind="ExternalInput")
    efT = nc.dram_tensor("efT", [128, E_PAD], BF16, kind="ExternalInput")
    rankT = nc.dram_tensor("rankT", [128, NT], F32, kind="ExternalInput")
    idx_rank = nc.dram_tensor("idx_rank", [128, E_PAD // 16], I16,
                              kind="ExternalInput")
    nfT_loc = nc.dram_tensor("nfT_loc", [128, NPC_PAD], BF16, kind="ExternalInput")
    cb_bf = nc.dram_tensor("cb_bf", [128, 5 * 128], BF16, kind="ExternalInput")
    cb_f32 = nc.dram_tensor("cb_f32", [128, 2 * 128 + 1], F32,
                            kind="ExternalInput")

    out_shard = nc.dram_tensor("out_shard", [NPC_PAD, 128], BF16,
                               kind="ExternalOutput")

    with tile.TileContext(nc) as tc:
        with (
            tc.tile_pool(name="consts", bufs=1) as consts,
            tc.tile_pool(name="ptiles", bufs=3) as ptiles,
            tc.tile_pool(name="estream", bufs=4) as estream,
            tc.tile_pool(name="gx", bufs=4) as gx,
            tc.tile_pool(name="ppsum", bufs=2, space="PSUM") as ppsum,
            tc.tile_pool(name="msgs", bufs=4) as msgs,
            tc.tile_pool(name="aggp", bufs=3, space="PSUM") as aggp,
            tc.tile_pool(name="wtiles", bufs=6) as wtiles,
        ):
            cbf = consts.tile([128, 5 * 128], BF16)
            cf32 = consts.tile([128, 2 * 128 + 1], F32)
            eps_sb = consts.tile([128, 1], F32)
            idxr_sb = consts.tile([128, E_PAD // 16], I16)
            rankT_sb = consts.tile([128, NT], F32)
            nfl_sb = consts.tile([128, NPC_PAD], BF16)
            p2bT = consts.tile([128, NPC_PAD], F32R)
            W1s = cbf[:, 0:128]
            W2s = cbf[:, 128:256]
            W3s = cbf[:, 256:384]
            iota_sb = cbf[:, 384:512]
            ident_sb = cbf[:, 512:640]
            lnw_sb = cf32[:, 0:128]
            lnb_sb = cf32[:, 128:256]
            bcol_sb = cf32[:, 256:257]

            # Startup choreography: the SP queue carries only what gates the
            # first window (consts blob, first p2bT chunk, head indices); the
            # Activation queue prefetches the first two windows' streams.
            half = (NPC_PAD // 1024) * 512
            head = min(int(tile_base[min(4, WPC - 1)]) * 8, E_PAD // 16)
            nc.sync.dma_start(out=cbf[:], in_=cb_bf[:])
            nc.sync.dma_start(out=nfl_sb[:, :half], in_=nfT_loc[:, :half])
            if head > 0:
                nc.sync.dma_start(out=idxr_sb[:, :head], in_=idx_rank[:, :head])
            nc.sync.dma_start(out=rankT_sb[:], in_=rankT[:])
            prefetched = {}
            for w in range(min(4, WPC)):
                twp = int(T_w[w])
                tbp = int(tile_base[w])
                if twp == 0:
                    continue
                ef_p = estream.tile([128, T_MAX * 128], BF16, tag="ef")
                nc.sync.dma_start(out=ef_p[:, :twp * 128],
                                  in_=efT[:, tbp * 128:(tbp + twp) * 128])
                nfx_p = estream.tile([128, T_MAX * 128], BF16, tag="nfx")
                nc.sync.dma_start(
                    out=nfx_p[:, :twp * 128],
                    in_=nf_expT[:, tbp * 128:(tbp + twp) * 128])
                prefetched[w] = (ef_p, nfx_p)
            nc.scalar.dma_start(out=nfl_sb[:, half:], in_=nfT_loc[:, half:])
            nc.scalar.dma_start(out=cf32[:], in_=cb_f32[:])
            nc.vector.memset(eps_sb[:], eps)

            # ---- phase B: p2bT = (nf_loc @ W2 + b).T, kept in SBUF ----------
            # nfT_loc stays resident: it also supplies the per-window residual.
            # Only the first chunk's matmuls run before window 0; the rest are
            # emitted at the window-1 boundary (PE executes in order).
            def emit_phase_b(j0_range):
                for j0 in j0_range:
                    k = min(512, NPC_PAD - j0)
                    psb = ppsum.tile([128, 8, 128], F32, tag="pp")
                    psb_flat = psb[:].rearrange("p a b -> p (a b)")
                    nc.tensor.matmul(out=psb_flat[:, :k], lhsT=W2s,
                                     rhs=nfl_sb[:, j0:j0 + k], start=True,
                                     stop=True)
                    nc.scalar.add(out=p2bT[:, j0:j0 + k], in_=psb_flat[:, :k],
                                  add=bcol_sb)
            emit_phase_b(range(0, half, 512))

            nc.sync.dma_start(out=rankT_sb[:], in_=rankT[:])

            # ---- edge loop --------------------------------------------------
            # remaining index chunks trickle out between the first windows so
            # no single DMA hold starves the stream prefetch
            idx_cols = E_PAD // 16
            n_chunks = 4
            chunk_edges = [head + (idx_cols - head) * i // n_chunks
                           for i in range(n_chunks + 1)]
            def flush_carry(carry):
                """Emit the previous window's last agg batch + its LN.

                The agg bank is drained to SBUF by the Activation engine
                immediately, so the PSUM bank recycles without waiting for
                the multi-pass DVE LayerNorm reads.
                """
                (pS4, pmsg, pk), agg, t_done, ptw, pw = carry
                for t in range(pk):
                    nc.tensor.matmul(
                        out=agg[:], lhsT=pS4[:, t, :], rhs=pmsg[:, t, :],
                        start=False, stop=(t_done == ptw - 1),
                        skip_group_check=True)
                    t_done += 1
                _emit_ln_store(nc, wtiles, agg[:], eps_sb, lnw_sb, lnb_sb,
                               out_shard, pw)

            carry = None
            n_win = WPC if max_windows is None else min(max_windows, WPC)
            for w in range(n_win):
                if 1 <= w <= 3:
                    seg = (NPC_PAD - half + 2) // 3
                    a0 = half + (w - 1) * seg
                    a1 = NPC_PAD if w == 3 else min(half + w * seg, NPC_PAD)
                    a0 = (a0 // 512) * 512
                    a1 = NPC_PAD if w == 3 else (a1 // 512) * 512
                    if a1 > a0:
                        emit_phase_b(range(a0, a1, 512))
                if w < n_chunks:
                    a, bnd = chunk_edges[w], chunk_edges[w + 1]
                    if bnd > a:
                        nc.sync.dma_start(out=idxr_sb[:, a:bnd],
                                          in_=idx_rank[:, a:bnd])
                tw = int(T_w[w])
                tb = int(tile_base[w])
                if tw == 0:
                    if carry is not None:
                        flush_carry(carry)
                        carry = None
                    agg = aggp.tile([128, 128], F32, tag="agg")
                    nc.tensor.matmul(
                        out=agg[:], lhsT=nfl_sb[:, w * 128:(w + 1) * 128],
                        rhs=ident_sb, start=True, stop=True,
                        skip_group_check=True)
                    _emit_ln_store(nc, wtiles, agg[:], eps_sb, lnw_sb, lnb_sb,
                                   out_shard, w)
                    continue

                if w in prefetched:
                    ef_sb, nfx_sb = prefetched.pop(w)
                else:
                    ef_sb = estream.tile([128, T_MAX * 128], BF16, tag="ef")
                    nc.sync.dma_start(out=ef_sb[:, :tw * 128],
                                      in_=efT[:, tb * 128:(tb + tw) * 128])
                    nfx_sb = estream.tile([128, T_MAX * 128], BF16, tag="nfx")
                    nc.sync.dma_start(
                        out=nfx_sb[:, :tw * 128],
                        in_=nf_expT[:, tb * 128:(tb + tw) * 128])
                agg = aggp.tile([128, 128], F32, tag="agg")
                # residual: agg starts as nf[window] via bf16 ident transpose
                nc.tensor.matmul(
                    out=agg[:], lhsT=nfl_sb[:, w * 128:(w + 1) * 128],
                    rhs=ident_sb, start=True, stop=False,
                    skip_group_check=True)
                # agg matmuls run one group behind the ps/relu pipeline so
                # the in-order PE never waits on the relu of the group it
                # just produced
                t_done = 0
                pending = None
                p2x = gx.tile([128, T_MAX * 128], F32R, tag="p2x")
                nc.gpsimd.ap_gather(
                    out_ap=p2x[:, :tw * 128].rearrange("p (n d) -> p n d", d=1),
                    in_ap=p2bT[:, w * 128:(w + 1) * 128].rearrange(
                        "p (n d) -> p n d", d=1),
                    idxs_ap=idxr_sb[:, tb * 8:(tb + tw) * 8],
                    channels=128, num_elems=128, d=1,
                    num_idxs=tw * 128)
                # bf16 view of p2x: odd lanes of the little-endian fp32 words
                # are the truncated-bf16 values -> 1 cyc/row PE transpose via
                # a normal matmul against the bf16 identity.
                p2x_bf = p2x[:].bitcast(BF16).rearrange(
                    "p (n two) -> p n two", two=2)
                for c0 in range(0, tw, 8):
                    k = min(8, tw - c0)
                    ps = ppsum.tile([128, 8, 128], F32, tag="pp")
                    S4 = msgs.tile([128, 8, 128], BF16, tag="S")
                    for t in range(k):
                        sl = slice((c0 + t) * 128, (c0 + t + 1) * 128)
                        nc.tensor.matmul(
                            out=ps[:, t, :],
                            lhsT=p2x_bf[:, sl, 1],
                            rhs=ident_sb,
                            start=True, stop=False, skip_group_check=True)
                        nc.tensor.matmul(
                            out=ps[:, t, :], lhsT=nfx_sb[:, sl], rhs=W1s,
                            start=False, stop=False, skip_group_check=True)
                        nc.tensor.matmul(
                            out=ps[:, t, :], lhsT=ef_sb[:, sl], rhs=W3s,
                            start=False, stop=True, skip_group_check=True)
                        nc.vector.tensor_scalar(
                            out=S4[:, t, :], in0=iota_sb,
                            scalar1=rankT_sb[:, tb + c0 + t:tb + c0 + t + 1],
                            scalar2=None, op0=is_eq)
                    msg = msgs.tile([128, 8, 128], BF16, tag="msg")
                    nc.scalar.activation(
                        out=msg[:, :k, :], in_=ps[:, :k, :],
                        func=mybir.ActivationFunctionType.Relu, scale=1.0)
                    if pending is not None:
                        pS4, pmsg, pk = pending
                        for t in range(pk):
                            nc.tensor.matmul(
                                out=agg[:], lhsT=pS4[:, t, :],
                                rhs=pmsg[:, t, :],
                                start=False, stop=False,
                                skip_group_check=True)
                            t_done += 1
                    if c0 == 0 and carry is not None:
                        flush_carry(carry)
                        carry = None
                    pending = (S4, msg, k)
                if carry is not None:
                    # window had a single group; flush late rather than drop
                    flush_carry(carry)
                carry = (pending, agg, t_done, tw, w)
            if carry is not None:
                flush_carry(carry)

    nc.compile()
    return nc


# ----------------------------------------------------------------------------
# Full entry: host prep + device run + assembly
# ----------------------------------------------------------------------------

def run(node_features, senders, receivers, edge_features, W, b, ln_w, ln_b,
        n_cores=8, return_nc=False):
    from concourse.bass_utils import run_bass_kernel_spmd
    st, in_maps = host_prep(node_features, senders, receivers, edge_features,
                            W, b, ln_w, ln_b, n_cores)
    nc = build_kernel(st)
    res = run_bass_kernel_spmd(nc, in_maps, core_ids=list(range(n_cores)))
    NPC_PAD = st["NPC_PAD"]
    stacked = np.concatenate(
        [np.asarray(res.results[c]["out_shard"], dtype=np.float32)
         for c in range(n_cores)], axis=0)
    rows = (st["node_core"] * NPC_PAD + st["node_slot"] * 128
            + st["node_rank"])
    out = stacked[rows]
    if return_nc:
        return out, nc, st, in_maps
    return out


# ----------------------------------------------------------------------------
# Harness entry point
# ----------------------------------------------------------------------------

def kernel(**inputs):
    """Full-input entry: shards across 8 NeuronCores internally."""
    out = run(
        node_features=np.asarray(inputs["node_features"], np.float32),
        senders=np.asarray(inputs["senders"], np.int32),
        receivers=np.asarray(inputs["receivers"], np.int32),
        edge_features=np.asarray(inputs["edge_features"], np.float32),
        W=np.asarray(inputs["W"], np.float32),
        b=np.asarray(inputs["b"], np.float32),
        ln_w=np.asarray(inputs["ln_w"], np.float32),
        ln_b=np.asarray(inputs["ln_b"], np.float32),
        n_cores=8,
    )
    return out.astype(np.float32)


# revision 106
# speedup vs baseline: 1.0086x; 1.0086x over previous
"""GNN message-passing kernel for Trainium2 (Bass/Tile), 8-core SPMD.

Sharding: edges sharded by receiver (edge/data parallel, no collectives),
with a degree-balanced node -> (core, window, rank) relabeling that minimizes
padded tile count. Host prep (pure indexing, no model FLOPs) expands
nf[senders] into a per-edge bf16 stream so the device never does a DRAM
gather on the sender side.

Per core, per 128-receiver window, per 128-edge tile (one PSUM group):
  ps  = transpose(p2x)            # P2b[r]: window-local ap_gather, then a
                                  # bf16-bitcast identity matmul (1 cyc/row)
      + nf_exp_tile.T @ W1        # bf16, host-expanded nf[senders]
      + ef_tile.T    @ W3         # bf16
  msg = relu(ps)                  # Activation engine, bf16 out
  S   = (iota == rank)            # one-hot, DVE tensor_scalar, bf16
  agg += S.T @ msg                # scatter-sum via matmul, fp32 PSUM
                                  # (agg matmuls pipelined one group behind)
  agg starts as nf[window] (residual) via a bf16 identity matmul;
  out = LayerNorm(agg) streamed out on the Activation DGE queue.
"""

import numpy as np
import ml_dtypes

import concourse.bacc as bacc
import concourse.tile as tile
import concourse.mybir as mybir
import concourse.bass as bass

F32 = mybir.dt.float32
# float32r crashes real TRN2 (walrus codegen bug for f32r weight loads) —
# keep the transpose path in plain fp32.
F32R = mybir.dt.float32
BF16 = mybir.dt.bfloat16
I16 = mybir.dt.int16

BF = ml_dtypes.bfloat16


# ----------------------------------------------------------------------------
# Host-side preparation (indexing / layout only — no model FLOPs)
# ----------------------------------------------------------------------------

def wrap_idx(arr):
    """int16 stream -> [128, L/16] wrapped layout (replicated per 16 rows)."""
    L = arr.shape[0]
    assert L % 16 == 0
    w16 = arr.reshape(-1, 16).T.astype(np.int16)   # [16, L/16]
    return np.ascontiguousarray(np.tile(w16, (8, 1)))


def host_prep(node_features, senders, receivers, edge_features, W, b, ln_w, ln_b,
              n_cores=8):
    N, H = node_features.shape
    E = senders.shape[0]
    assert H == 128
    NPC = N // n_cores                      # nodes per core
    WPC = (NPC + 127) // 128                # windows per core
    NPC_PAD = WPC * 128

    node_features = np.asarray(node_features, np.float32)
    senders = np.asarray(senders, np.int32)
    receivers = np.asarray(receivers, np.int32)
    edge_features = np.asarray(edge_features, np.float32)

    # ---- degree-balanced node -> (core, window-slot, rank) assignment ------
    # Nodes are packed into n_cores*WPC windows of <=128 nodes so that window
    # degree sums are near-equal (greedy, highest degree first), then windows
    # with similar sums are dealt to the same slot across cores.  This
    # minimizes sum_w max_c ceil(cnt/128), i.e. the padded tile count.
    import heapq
    deg = np.bincount(receivers, minlength=N).astype(np.int64)
    W_TOT = n_cores * WPC
    win_of_node = np.empty(N, np.int64)
    nfill = np.zeros(W_TOT, np.int64)
    heap = [(0, wi) for wi in range(W_TOT)]
    heapq.heapify(heap)
    for nd in np.argsort(-deg, kind="stable"):
        s, wi = heapq.heappop(heap)
        win_of_node[nd] = wi
        nfill[wi] += 1
        if nfill[wi] < 128:
            heapq.heappush(heap, (s + int(deg[nd]), wi))
    wsum = np.bincount(win_of_node, weights=deg.astype(np.float64),
                       minlength=W_TOT)
    wsorted = np.argsort(-wsum, kind="stable")
    slot_of_win = np.empty(W_TOT, np.int64)
    core_of_win = np.empty(W_TOT, np.int64)
    slot_of_win[wsorted] = np.arange(W_TOT) // n_cores
    core_of_win[wsorted] = np.arange(W_TOT) % n_cores
    # rank of node within its window
    order_nd = np.argsort(win_of_node, kind="stable")
    rank_of_node = np.empty(N, np.int64)
    wo = win_of_node[order_nd]
    starts = np.concatenate(([0], np.nonzero(np.diff(wo))[0] + 1))
    grp_start = np.repeat(starts, np.diff(np.concatenate((starts, [N]))))
    rank_of_node[order_nd] = np.arange(N) - grp_start

    node_core = core_of_win[win_of_node]
    node_slot = slot_of_win[win_of_node]
    node_rank = rank_of_node

    # ---- pass 1: per-core window counts ------------------------------------
    core_of_edge = node_core[receivers]
    per_core = []
    cnt = np.zeros((n_cores, WPC), np.int64)
    for c in range(n_cores):
        sel = np.nonzero(core_of_edge == c)[0]
        w = node_slot[receivers[sel]]
        order = np.argsort(w, kind="stable")
        sel, w = sel[order], w[order]
        cnt[c] = np.bincount(w, minlength=WPC)
        per_core.append((sel, w))

    T_w = ((cnt.max(axis=0) + 127) // 128).astype(np.int64)  # shared all cores
    NT = int(T_w.sum())
    E_PAD = NT * 128
    tile_base = np.concatenate(([0], np.cumsum(T_w)[:-1]))

    structure = dict(N=N, H=H, E=E, NPC=NPC, WPC=WPC, NPC_PAD=NPC_PAD,
                     NT=NT, E_PAD=E_PAD, T_w=T_w, tile_base=tile_base,
                     node_core=node_core, node_slot=node_slot,
                     node_rank=node_rank)

    iota_row = np.broadcast_to(np.arange(128, dtype=np.float32),
                               (128, 128)).astype(BF)
    # bf16 const blob: W1 | W2 | W3 | iota | ident   [128, 5*128]
    cb_bf = np.concatenate([
        np.ascontiguousarray(W[0:128]).astype(BF),
        np.ascontiguousarray(W[128:256]).astype(BF),
        np.ascontiguousarray(W[256:384]).astype(BF),
        iota_row,
        np.eye(128, dtype=np.float32).astype(BF),
    ], axis=1)
    # f32 const blob: lnw | lnb | b_col   [128, 2*128+1]
    cb_f32 = np.concatenate([
        np.broadcast_to(np.asarray(ln_w, np.float32), (128, 128)),
        np.broadcast_to(np.asarray(ln_b, np.float32), (128, 128)),
        np.asarray(b, np.float32).reshape(128, 1),
    ], axis=1)
    shared = {
        "cb_bf": np.ascontiguousarray(cb_bf),
        "cb_f32": np.ascontiguousarray(cb_f32),
    }

    # ---- pass 2: per-core padded streams ------------------------------------
    in_maps = []
    for c in range(n_cores):
        sel, w = per_core[c]
        Ec = sel.shape[0]
        starts = np.concatenate(([0], np.nonzero(np.diff(w))[0] + 1))
        grp_start = np.repeat(starts, np.diff(np.concatenate((starts, [Ec]))))
        j = np.arange(Ec) - grp_start
        pos = tile_base[w] * 128 + j
        rank = node_rank[receivers[sel]]

        nf_exp = np.zeros((E_PAD, 128), np.float32)
        nf_exp[pos] = node_features[senders[sel]]
        ef_pad = np.zeros((E_PAD, 128), np.float32)
        ef_pad[pos] = edge_features[sel]

        rank_arr = np.full(E_PAD, -1.0, np.float32)
        rank_arr[pos] = rank
        rankT = np.ascontiguousarray(rank_arr.reshape(NT, 128).T)   # [128, NT]

        idx_rank = np.zeros(E_PAD, np.int64)
        idx_rank[pos] = rank

        # nodes of this core, laid out by (slot, rank), feature-major
        mine = np.nonzero(node_core == c)[0]
        rows = node_slot[mine] * 128 + node_rank[mine]
        nfT_loc = np.zeros((128, NPC_PAD), np.float32)
        nfT_loc[:, rows] = node_features[mine].T

        m = dict(shared)
        m.update({
            "nf_expT": np.ascontiguousarray(nf_exp.T).astype(BF),
            "efT": np.ascontiguousarray(ef_pad.T).astype(BF),
            "rankT": rankT,
            "idx_rank": wrap_idx(idx_rank),
            "nfT_loc": np.ascontiguousarray(nfT_loc).astype(BF),
        })
        in_maps.append(m)

    return structure, in_maps


# ----------------------------------------------------------------------------
# Bass kernel builder
# ----------------------------------------------------------------------------

def _emit_ln_store(nc, wtiles, x, eps_sb, lnw_sb, lnb_sb, out_shard, w):
    """LayerNorm(x) * ln_w + ln_b -> out_shard[w*128:(w+1)*128].

    The store goes out on the Activation DGE queue so it cannot head-of-line
    block the SP queue that prefetches the next windows' edge streams.
    """
    stats = wtiles.tile([128, 6], F32, tag="stats")
    nc.vector.bn_stats(out=stats[:], in_=x[:])
    mv = wtiles.tile([128, 2], F32, tag="mv")
    nc.vector.bn_aggr(out=mv[:], in_=stats[:])
    sd = wtiles.tile([128, 1], F32, tag="sd")
    nc.scalar.activation(
        out=sd[:], in_=mv[:, 1:2],
        func=mybir.ActivationFunctionType.Sqrt,
        bias=eps_sb[:], scale=1.0)
    rs = wtiles.tile([128, 1], F32, tag="rs")
    nc.vector.reciprocal(out=rs[:], in_=sd[:])
    xn = wtiles.tile([128, 128], F32, tag="xn")
    nc.vector.tensor_scalar(
        out=xn[:], in0=x[:], scalar1=mv[:, 0:1], scalar2=rs[:],
        op0=mybir.AluOpType.subtract, op1=mybir.AluOpType.mult)
    xw = wtiles.tile([128, 128], F32, tag="xw")
    nc.vector.tensor_mul(out=xw[:], in0=xn[:], in1=lnw_sb)
    ot = wtiles.tile([128, 128], BF16, tag="ot")
    nc.vector.tensor_add(out=ot[:], in0=xw[:], in1=lnb_sb)
    nc.scalar.dma_start(out=out_shard[w * 128:(w + 1) * 128, :], in_=ot[:])


def build_kernel(st, eps=1e-5, max_windows=None):
    NPC_PAD, WPC = st["NPC_PAD"], st["WPC"]
    NT, E_PAD = st["NT"], st["E_PAD"]
    T_w, tile_base = st["T_w"], st["tile_base"]
    T_MAX = int(T_w.max())
    is_eq = mybir.AluOpType.is_equal

    nc = bacc.Bacc("TRN2", target_bir_lowering=False, debug=False)

    # inputs
    nf_expT = nc.dram_tensor("nf_expT", [128, E_PAD], BF16, kind="ExternalInput")
    efT = nc.dram_tensor("efT", [128, E_PAD], BF16, kind="ExternalInput")
    rankT = nc.dram_tensor("rankT", [128, NT], F32, kind="ExternalInput")
    idx_rank = nc.dram_tensor("idx_rank", [128, E_PAD // 16], I16,
                              kind="ExternalInput")
    nfT_loc = nc.dram_tensor("nfT_loc", [128, NPC_PAD], BF16, kind="ExternalInput")
    cb_bf = nc.dram_tensor("cb_bf", [128, 5 * 128], BF16, kind="ExternalInput")
    cb_f32 = nc.dram_tensor("cb_f32", [128, 2 * 128 + 1], F32,
                            kind="ExternalInput")

    out_shard = nc.dram_tensor("out_shard", [NPC_PAD, 128], BF16,
                               kind="ExternalOutput")

    with tile.TileContext(nc) as tc:
        with (
            tc.tile_pool(name="consts", bufs=1) as consts,
            tc.tile_pool(name="ptiles", bufs=3) as ptiles,
            tc.tile_pool(name="estream", bufs=4) as estream,
            tc.tile_pool(name="gx", bufs=4) as gx,
            tc.tile_pool(name="ppsum", bufs=2, space="PSUM") as ppsum,
            tc.tile_pool(name="msgs", bufs=4) as msgs,
            tc.tile_pool(name="aggp", bufs=3, space="PSUM") as aggp,
            tc.tile_pool(name="wtiles", bufs=6) as wtiles,
        ):
            cbf = consts.tile([128, 5 * 128], BF16)
            cf32 = consts.tile([128, 2 * 128 + 1], F32)
            eps_sb = consts.tile([128, 1], F32)
            idxr_sb = consts.tile([128, E_PAD // 16], I16)
            rankT_sb = consts.tile([128, NT], F32)
            nfl_sb = consts.tile([128, NPC_PAD], BF16)
            p2bT = consts.tile([128, NPC_PAD], F32R)
            W1s = cbf[:, 0:128]
            W2s = cbf[:, 128:256]
            W3s = cbf[:, 256:384]
            iota_sb = cbf[:, 384:512]
            ident_sb = cbf[:, 512:640]
            lnw_sb = cf32[:, 0:128]
            lnb_sb = cf32[:, 128:256]
            bcol_sb = cf32[:, 256:257]

            # Startup choreography: the SP queue carries only what gates the
            # first window (consts blob, first p2bT chunk, head indices); the
            # Activation queue prefetches the first two windows' streams.
            half = (NPC_PAD // 1024) * 512
            head = min(int(tile_base[min(4, WPC - 1)]) * 8, E_PAD // 16)
            nc.sync.dma_start(out=cbf[:], in_=cb_bf[:])
            nc.sync.dma_start(out=nfl_sb[:, :half], in_=nfT_loc[:, :half])
            if head > 0:
                nc.sync.dma_start(out=idxr_sb[:, :head], in_=idx_rank[:, :head])
            nc.sync.dma_start(out=rankT_sb[:], in_=rankT[:])
            prefetched = {}
            for w in range(min(3, WPC)):
                twp = int(T_w[w])
                tbp = int(tile_base[w])
                if twp == 0:
                    continue
                ef_p = estream.tile([128, T_MAX * 128], BF16, tag="ef")
                nc.sync.dma_start(out=ef_p[:, :twp * 128],
                                  in_=efT[:, tbp * 128:(tbp + twp) * 128])
                nfx_p = estream.tile([128, T_MAX * 128], BF16, tag="nfx")
                nc.sync.dma_start(
                    out=nfx_p[:, :twp * 128],
                    in_=nf_expT[:, tbp * 128:(tbp + twp) * 128])
                prefetched[w] = (ef_p, nfx_p)
            nc.scalar.dma_start(out=nfl_sb[:, half:], in_=nfT_loc[:, half:])
            nc.scalar.dma_start(out=cf32[:], in_=cb_f32[:])
            nc.vector.memset(eps_sb[:], eps)

            # ---- phase B: p2bT = (nf_loc @ W2 + b).T, kept in SBUF ----------
            # nfT_loc stays resident: it also supplies the per-window residual.
            # Only the first chunk's matmuls run before window 0; the rest are
            # emitted at the window-1 boundary (PE executes in order).
            def emit_phase_b(j0_range):
                for j0 in j0_range:
                    k = min(512, NPC_PAD - j0)
                    psb = ppsum.tile([128, 8, 128], F32, tag="pp")
                    psb_flat = psb[:].rearrange("p a b -> p (a b)")
                    nc.tensor.matmul(out=psb_flat[:, :k], lhsT=W2s,
                                     rhs=nfl_sb[:, j0:j0 + k], start=True,
                                     stop=True)
                    nc.scalar.add(out=p2bT[:, j0:j0 + k], in_=psb_flat[:, :k],
                                  add=bcol_sb)
            emit_phase_b(range(0, half, 512))

            nc.sync.dma_start(out=rankT_sb[:], in_=rankT[:])

            # ---- edge loop --------------------------------------------------
            # remaining index chunks trickle out between the first windows so
            # no single DMA hold starves the stream prefetch
            idx_cols = E_PAD // 16
            n_chunks = 4
            chunk_edges = [head + (idx_cols - head) * i // n_chunks
                           for i in range(n_chunks + 1)]
            def flush_carry(carry):
                """Emit the previous window's last agg batch + its LN.

                The agg bank is drained to SBUF by the Activation engine
                immediately, so the PSUM bank recycles without waiting for
                the multi-pass DVE LayerNorm reads.
                """
                (pS4, pmsg, pk), agg, t_done, ptw, pw = carry
                for t in range(pk):
                    nc.tensor.matmul(
                        out=agg[:], lhsT=pS4[:, t, :], rhs=pmsg[:, t, :],
                        start=False, stop=(t_done == ptw - 1),
                        skip_group_check=True)
                    t_done += 1
                _emit_ln_store(nc, wtiles, agg[:], eps_sb, lnw_sb, lnb_sb,
                               out_shard, pw)

            carry = None
            n_win = WPC if max_windows is None else min(max_windows, WPC)
            for w in range(n_win):
                if 1 <= w <= 3:
                    seg = (NPC_PAD - half + 2) // 3
                    a0 = half + (w - 1) * seg
                    a1 = NPC_PAD if w == 3 else min(half + w * seg, NPC_PAD)
                    a0 = (a0 // 512) * 512
                    a1 = NPC_PAD if w == 3 else (a1 // 512) * 512
                    if a1 > a0:
                        emit_phase_b(range(a0, a1, 512))
                if w < n_chunks:
                    a, bnd = chunk_edges[w], chunk_edges[w + 1]
                    if bnd > a:
                        nc.sync.dma_start(out=idxr_sb[:, a:bnd],
                                          in_=idx_rank[:, a:bnd])
                tw = int(T_w[w])
                tb = int(tile_base[w])
                if tw == 0:
                    if carry is not None:
                        flush_carry(carry)
                        carry = None
                    agg = aggp.tile([128, 128], F32, tag="agg")
                    nc.tensor.matmul(
                        out=agg[:], lhsT=nfl_sb[:, w * 128:(w + 1) * 128],
                        rhs=ident_sb, start=True, stop=True,
                        skip_group_check=True)
                    _emit_ln_store(nc, wtiles, agg[:], eps_sb, lnw_sb, lnb_sb,
                                   out_shard, w)
                    continue

                if w in prefetched:
                    ef_sb, nfx_sb = prefetched.pop(w)
                else:
                    ef_sb = estream.tile([128, T_MAX * 128], BF16, tag="ef")
                    nc.sync.dma_start(out=ef_sb[:, :tw * 128],
                                      in_=efT[:, tb * 128:(tb + tw) * 128])
                    nfx_sb = estream.tile([128, T_MAX * 128], BF16, tag="nfx")
                    nc.sync.dma_start(
                        out=nfx_sb[:, :tw * 128],
                        in_=nf_expT[:, tb * 128:(tb + tw) * 128])
                agg = aggp.tile([128, 128], F32, tag="agg")
                # residual: agg starts as nf[window] via bf16 ident transpose
                nc.tensor.matmul(
                    out=agg[:], lhsT=nfl_sb[:, w * 128:(w + 1) * 128],
                    rhs=ident_sb, start=True, stop=False,
                    skip_group_check=True)
                # agg matmuls run one group behind the ps/relu pipeline so
                # the in-order PE never waits on the relu of the group it
                # just produced
                t_done = 0
                pending = None
                p2x = gx.tile([128, T_MAX * 128], F32R, tag="p2x")
                nc.gpsimd.ap_gather(
                    out_ap=p2x[:, :tw * 128].rearrange("p (n d) -> p n d", d=1),
                    in_ap=p2bT[:, w * 128:(w + 1) * 128].rearrange(
                        "p (n d) -> p n d", d=1),
                    idxs_ap=idxr_sb[:, tb * 8:(tb + tw) * 8],
                    channels=128, num_elems=128, d=1,
                    num_idxs=tw * 128)
                # bf16 view of p2x: odd lanes of the little-endian fp32 words
                # are the truncated-bf16 values -> 1 cyc/row PE transpose via
                # a normal matmul against the bf16 identity.
                p2x_bf = p2x[:].bitcast(BF16).rearrange(
                    "p (n two) -> p n two", two=2)
                for c0 in range(0, tw, 8):
                    k = min(8, tw - c0)
                    ps = ppsum.tile([128, 8, 128], F32, tag="pp")
                    S4 = msgs.tile([128, 8, 128], BF16, tag="S")
                    for t in range(k):
                        sl = slice((c0 + t) * 128, (c0 + t + 1) * 128)
                        nc.tensor.matmul(
                            out=ps[:, t, :],
                            lhsT=p2x_bf[:, sl, 1],
                            rhs=ident_sb,
                            start=True, stop=False, skip_group_check=True)
                        nc.tensor.matmul(
                            out=ps[:, t, :], lhsT=nfx_sb[:, sl], rhs=W1s,
                            start=False, stop=False, skip_group_check=True)
                        nc.tensor.matmul(
                            out=ps[:, t, :], lhsT=ef_sb[:, sl], rhs=W3s,
                            start=False, stop=True, skip_group_check=True)
                        nc.vector.tensor_scalar(
                            out=S4[:, t, :], in0=iota_sb,
                            scalar1=rankT_sb[:, tb + c0 + t:tb + c0 + t + 1],
                            scalar2=None, op0=is_eq)
                    msg = msgs.tile([128, 8, 128], BF16, tag="msg")
                    nc.scalar.activation(
                        out=msg[:, :k, :], in_=ps[:, :k, :],
                        func=mybir.ActivationFunctionType.Relu, scale=1.0)
                    if pending is not None:
                        pS4, pmsg, pk = pending
                        for t in range(pk):
                            nc.tensor.matmul(
                                out=agg[:], lhsT=pS4[:, t, :],
                                rhs=pmsg[:, t, :],
                                start=False, stop=False,
                                skip_group_check=True)
                            t_done += 1
                    if c0 == 0 and carry is not None:
                        flush_carry(carry)
                        carry = None
                    pending = (S4, msg, k)
                if carry is not None:
                    # window had a single group; flush late rather than drop
                    flush_carry(carry)
                carry = (pending, agg, t_done, tw, w)
            if carry is not None:
                flush_carry(carry)

    nc.compile()
    return nc


# ----------------------------------------------------------------------------
# Full entry: host prep + device run + assembly
# ----------------------------------------------------------------------------

def run(node_features, senders, receivers, edge_features, W, b, ln_w, ln_b,
        n_cores=8, return_nc=False):
    from concourse.bass_utils import run_bass_kernel_spmd
    st, in_maps = host_prep(node_features, senders, receivers, edge_features,
                            W, b, ln_w, ln_b, n_cores)
    nc = build_kernel(st)
    res = run_bass_kernel_spmd(nc, in_maps, core_ids=list(range(n_cores)))
    NPC_PAD = st["NPC_PAD"]
    stacked = np.concatenate(
        [np.asarray(res.results[c]["out_shard"], dtype=np.float32)
         for c in range(n_cores)], axis=0)
    rows = (st["node_core"] * NPC_PAD + st["node_slot"] * 128
            + st["node_rank"])
    out = stacked[rows]
    if return_nc:
        return out, nc, st, in_maps
    return out


# ----------------------------------------------------------------------------
# Harness entry point
# ----------------------------------------------------------------------------

def kernel(**inputs):
    """Full-input entry: shards across 8 NeuronCores internally."""
    out = run(
        node_features=np.asarray(inputs["node_features"], np.float32),
        senders=np.asarray(inputs["senders"], np.int32),
        receivers=np.asarray(inputs["receivers"], np.int32),
        edge_features=np.asarray(inputs["edge_features"], np.float32),
        W=np.asarray(inputs["W"], np.float32),
        b=np.asarray(inputs["b"], np.float32),
        ln_w=np.asarray(inputs["ln_w"], np.float32),
        ln_b=np.asarray(inputs["ln_b"], np.float32),
        n_cores=8,
    )
    return out.astype(np.float32)
